# revision 1
# baseline (speedup 1.0000x reference)
"""ConvLSTM (2-layer, HID=64, 64x64, T=16, B=16) Trainium2 Bass kernel.

Sharding: data-parallel over batch B=16 -> 2 per NeuronCore across 8 cores;
weights/biases replicated; the sequential T-loop runs locally per core.

Per core the 3x3 convs are computed as 9 shift-offset matmuls (K=Cin,
M=128 out-channels per PSUM tile, N=512 spatial) accumulating in PSUM, in
float32r (full PE rate, ~1e-4 end-to-end error). States live in SBUF for
the whole kernel:
  inp0 [65, 2, 66, 66]  f32r : p0 = x(t) padded, p1-64 = h0(t-1)  (cell0 rhs, K=65)
  inp1 [128, 2, 66, 66] f32r : p0-63 = h0(t), p64-127 = h1        (cell1 rhs, K=128)
  c0t/c1t [128, 2, 64, 64] f32 : [64:128] = c  (base-64 alignment with f-gate)
Gates: PSUM -> ScalarE sigmoid/tanh (conv bias fused via ACT bias) -> VectorE
state update; partition-base offsets align gate/state lanes.

Within a cell, all conv matmuls are emitted before any state write so
Tile's program-order dependency tracking sees the in-place h updates
correctly; single-row seam overlaps (which Tile's subtile tracker misses)
get explicit dependency edges.
"""
import numpy as np
import concourse.tile as tile
from concourse import mybir, bacc
from concourse.bass import _add_dep_helper
from concourse.bass_utils import run_bass_kernel_spmd

F32 = mybir.dt.float32
F32R = mybir.dt.float32r
SIG = mybir.ActivationFunctionType.Sigmoid
TANH = mybir.ActivationFunctionType.Tanh
RELU = mybir.ActivationFunctionType.Relu

N_CORES = 8
B_LOC = 2
H = W = 64
HP = WP = 66
EG_ROWS = 16
CH_ROWS = 8
N_EG = H // EG_ROWS
N_MM = CH_ROWS * W  # 512


def _build(T=16):
    nc = bacc.Bacc("TRN2", target_bir_lowering=False, debug=False, num_devices=N_CORES)

    x_d = nc.dram_tensor("xp", [T, 1, B_LOC, HP, WP], F32R, kind="ExternalInput").ap()
    w0_d = nc.dram_tensor("w0t", [65, 2, 9, 128], F32R, kind="ExternalInput").ap()
    w1_d = nc.dram_tensor("w1t", [128, 2, 9, 128], F32R, kind="ExternalInput").ap()
    b_d = nc.dram_tensor("bt", [128, 4], F32, kind="ExternalInput").ap()
    wh_d = nc.dram_tensor("wht", [128, 1], F32R, kind="ExternalInput").ap()
    bh_d = nc.dram_tensor("bht", [1, 1], F32, kind="ExternalInput").ap()
    y_d = nc.dram_tensor("y", [B_LOC, H * W], F32, kind="ExternalOutput").ap()

    with tile.TileContext(nc) as tc:
        with tc.tile_pool(name="state", bufs=1) as state, \
                tc.tile_pool(name="work", bufs=2) as work, \
                tc.tile_pool(name="psp", bufs=2, space="PSUM") as psp:
            inp0 = state.tile([65, B_LOC, HP, WP], F32R)
            inp1 = state.tile([128, B_LOC, HP, WP], F32R)
            c0t = state.tile([128, B_LOC, H, W], F32)
            c1t = state.tile([128, B_LOC, H, W], F32)
            w0t = state.tile([65, 2, 9, 128], F32R)
            w1t = state.tile([128, 2, 9, 128], F32R)
            b_sb = state.tile([128, 4], F32)
            whT = state.tile([128, 1], F32R)
            bh_sb = state.tile([1, 1], F32)

            nc.sync.dma_start(out=w0t, in_=w0_d)
            nc.sync.dma_start(out=w1t, in_=w1_d)
            nc.sync.dma_start(out=b_sb, in_=b_d)
            nc.sync.dma_start(out=whT, in_=wh_d)
            nc.sync.dma_start(out=bh_sb, in_=bh_d)

            nc.vector.memset(inp0.bitcast(mybir.dt.uint32), 0)
            nc.vector.memset(inp1.bitcast(mybir.dt.uint32), 0)
            nc.vector.memset(c0t[64:128], 0.0)
            nc.vector.memset(c1t[64:128], 0.0)

            nc.sync.dma_start(out=inp0[0:1], in_=x_d[0])

            h0w = {}
            h1w = {}
            mm_info = {}

            def do_cell(cell, t):
                if cell == 0:
                    rhs_t, K, wt, ct = inp0, 65, w0t, c0t
                    h_dst, hw_d = inp1[0:64], h0w
                else:
                    rhs_t, K, wt, ct = inp1, 128, w1t, c1t
                    h_dst, hw_d = inp1[64:128], h1w
                bcol = 2 * cell
                psums = {}
                # conv phase: all matmuls before any state write
                for b in range(B_LOC):
                    for eg in range(N_EG):
                        if cell == 1:
                            info = mm_info.setdefault((t, b, eg), dict(lasts=[], dn=[], up=[]))
                        p_if = psp.tile([128, 2, N_MM], F32, tag="pif", name=f"pif_{t}_{cell}_{b}_{eg}")
                        p_og = psp.tile([128, 2, N_MM], F32, tag="pog", name=f"pog_{t}_{cell}_{b}_{eg}")
                        psums[(b, eg)] = (p_if, p_og)
                        for half in range(2):
                            r0 = eg * EG_ROWS + half * CH_ROWS
                            for off in range(9):
                                dy, dx = off // 3, off % 3
                                rhs = rhs_t[0:K, b, r0 + dy:r0 + dy + CH_ROWS, dx:dx + W]
                                st, sp = off == 0, off == 8
                                mi = nc.tensor.matmul(p_if[:, half], lhsT=wt[:, 0, off], rhs=rhs,
                                                      start=st, stop=sp)
                                mo = nc.tensor.matmul(p_og[:, half], lhsT=wt[:, 1, off], rhs=rhs,
                                                      start=st, stop=sp)
                                if cell == 1:
                                    if sp:
                                        info["lasts"] += [mi.ins, mo.ins]
                                    if half == 0 and dy == 0 and (t, b, eg - 1) in h0w:
                                        for mm in (mi, mo):
                                            _add_dep_helper(mm.ins, h0w[(t, b, eg - 1)], reason="h0 seam RAW dn")
                                    if half == 1 and dy == 2 and (t, b, eg + 1) in h0w:
                                        for mm in (mi, mo):
                                            _add_dep_helper(mm.ins, h0w[(t, b, eg + 1)], reason="h0 seam RAW up")
                # elementwise phase
                for b in range(B_LOC):
                    for eg in range(N_EG):
                        p_if, p_og = psums[(b, eg)]
                        pif_f = p_if.rearrange("p a b -> p (a b)")
                        pog_f = p_og.rearrange("p a b -> p (a b)")
                        NE = 2 * N_MM
                        if_h = work.tile([128, NE], F32, tag="ifh", name=f"ifh_{t}_{cell}_{b}_{eg}")
                        g_h = work.tile([64, NE], F32, tag="gh", name=f"gh_{t}_{cell}_{b}_{eg}")
                        o_h = work.tile([64, NE], F32, tag="oh", name=f"oh_{t}_{cell}_{b}_{eg}")
                        m1 = work.tile([128, NE], F32, tag="m1", name=f"m1_{t}_{cell}_{b}_{eg}")
                        m2 = work.tile([128, NE], F32, tag="m2", name=f"m2_{t}_{cell}_{b}_{eg}")
                        t5 = work.tile([64, NE], F32, tag="t5", name=f"t5_{t}_{cell}_{b}_{eg}")
                        nc.scalar.activation(out=if_h, in_=pif_f, func=SIG,
                                             bias=b_sb[:, bcol:bcol + 1])
                        nc.scalar.activation(out=g_h, in_=pog_f[64:128], func=TANH,
                                             bias=b_sb[64:128, bcol + 1:bcol + 2])
                        nc.scalar.activation(out=o_h, in_=pog_f[0:64], func=SIG,
                                             bias=b_sb[0:64, bcol + 1:bcol + 2])
                        cseg = ct[64:128, b].rearrange("p a b -> p (a b)")[:, eg * EG_ROWS * W:(eg + 1) * EG_ROWS * W]
                        nc.vector.tensor_mul(m1[64:128], if_h[64:128], cseg)
                        nc.vector.tensor_mul(m2[64:128], if_h[0:64], g_h)
                        nc.vector.tensor_add(cseg, m1[64:128], m2[64:128])
                        nc.scalar.activation(out=t5, in_=cseg, func=TANH)
                        hseg = h_dst[:, b, 1 + eg * EG_ROWS:1 + (eg + 1) * EG_ROWS, 1:1 + W]
                        hw = nc.vector.tensor_mul(hseg, o_h, t5)
                        hw_d[(t, b, eg)] = hw.ins
                        if cell == 1:
                            for dg in (-1, 1):
                                if (t, b, eg + dg) in mm_info:
                                    for mm in mm_info[(t, b, eg + dg)]["lasts"]:
                                        _add_dep_helper(hw.ins, mm, reason="h1 seam WAR")

            for t in range(T):
                do_cell(0, t)
                nc.sync.dma_start(out=inp0[1:65], in_=inp1[0:64])
                if t + 1 < T:
                    nc.sync.dma_start(out=inp0[0:1], in_=x_d[t + 1])
                do_cell(1, t)

            for b in range(B_LOC):
                for ch in range(H // CH_ROWS):
                    p_h = psp.tile([1, N_MM], F32, tag="pif", name=f"ph_{b}_{ch}")
                    rhs = inp1[64:128, b, 1 + ch * CH_ROWS:1 + (ch + 1) * CH_ROWS, 1:1 + W]
                    mh = nc.tensor.matmul(p_h, lhsT=whT[64:128], rhs=rhs, start=True, stop=True)
                    if (T - 1, b, ch // 2) in h1w:
                        _add_dep_helper(mh.ins, h1w[(T - 1, b, ch // 2)], reason="head RAW")
                    h_out = work.tile([1, N_MM], F32, tag="ho", name=f"ho_{b}_{ch}")
                    nc.scalar.activation(out=h_out, in_=p_h, func=RELU, bias=bh_sb[0:1, 0:1])
                    nc.sync.dma_start(out=y_d[b:b + 1, ch * N_MM:(ch + 1) * N_MM], in_=h_out)

    nc.compile()
    return nc


def _prep_inputs(x, w0, b0, w1, b1, wh, bh):
    x = np.asarray(x, np.float32)
    B, T = x.shape[0], x.shape[1]
    bl = B // N_CORES

    def wprep(w, K):
        wt = np.asarray(w, np.float32).reshape(2, 128, K, 3, 3)
        wt = np.transpose(wt, (2, 0, 3, 4, 1))
        return np.ascontiguousarray(wt.reshape(K, 2, 9, 128))

    w0t = wprep(w0, 65)
    w1t = wprep(w1, 128)
    b0 = np.asarray(b0, np.float32)
    b1 = np.asarray(b1, np.float32)
    bt = np.stack([b0[0:128], b0[128:256], b1[0:128], b1[128:256]], axis=1).astype(np.float32)
    wht = np.zeros((128, 1), np.float32)
    wht[64:128, 0] = np.asarray(wh, np.float32).reshape(64)
    bht = np.array([[float(np.asarray(bh).reshape(-1)[0])]], np.float32)

    xp_all = np.zeros((B, T, 1, HP, WP), np.float32)
    xp_all[:, :, 0, 1:1 + H, 1:1 + W] = x[:, :, 0]

    in_maps = []
    for c in range(N_CORES):
        xp = np.ascontiguousarray(xp_all[c * bl:(c + 1) * bl].transpose(1, 2, 0, 3, 4))
        in_maps.append({"xp": xp, "w0t": w0t, "w1t": w1t, "bt": bt,
                        "wht": wht, "bht": bht})
    return in_maps


_NC_CACHE = {}


def kernel(x, w0, b0, w1, b1, wh, bh):
    x = np.asarray(x)
    B, T = x.shape[0], x.shape[1]
    if T not in _NC_CACHE:
        _NC_CACHE[T] = _build(T=T)
    nc = _NC_CACHE[T]
    in_maps = _prep_inputs(x, w0, b0, w1, b1, wh, bh)
    res = run_bass_kernel_spmd(nc, in_maps, core_ids=list(range(N_CORES)))
    bl = B // N_CORES
    out = np.zeros((B, 1, H, W), np.float32)
    for c, r in enumerate(res.results):
        out[c * bl:(c + 1) * bl, 0] = r["y"].reshape(bl, H, W)
    return out



# revision 4
# speedup vs baseline: 1.4996x; 1.4996x over previous
"""ConvLSTM (2-layer, HID=64, 64x64, T=16, B=16) Trainium2 Bass kernel.

Sharding: data-parallel over batch B=16 -> 2 per NeuronCore across 8 cores;
weights/biases replicated; the sequential T-loop runs locally per core.

v2: bf16 matmuls (PSUM f32 accumulate) + cell0 shift-offset K-packing.
Cell0's 3x3 conv (K=65: x + 64 h-ch) runs as 6 matmuls per PSUM bank-pass
instead of 9:
  - 1x K=73: x(t) im2col'd into 9 partitions + h0 pre-shifted by (2,2)
  - 3x K=128 offset-pairs (dy,0)+(dy,1) against [h0 ; h0 shifted by (0,1)]
  - 2x K=64 singles (dy,2), dy=0,1
The extra shifted h0 copies are SBUF->SBUF DMAs overlapped under cell1's
conv. Cell1 (K=128, 9 offsets) is already perfectly packed: 9 matmuls.

Elementwise uses a fused layout: tanh(g) is written by ScalarE directly into
partitions 0-63 above the c state (partitions 64-127) at the same free
offset, so one 128-partition VectorE mul yields both i*tanh(g) and f*c
(3 DVE ops per chunk instead of 4).

Within a cell, all conv matmuls are emitted before any state write so
Tile's program-order dependency tracking sees the in-place h updates
correctly; single-row seam overlaps (which Tile's subtile tracker misses)
get explicit dependency edges.
"""
import numpy as np
import ml_dtypes
import concourse.tile as tile
from concourse import mybir, bacc
from concourse.bass import _add_dep_helper
from concourse.bass_utils import run_bass_kernel_spmd

F32 = mybir.dt.float32
BF16 = mybir.dt.bfloat16
SIG = mybir.ActivationFunctionType.Sigmoid
TANH = mybir.ActivationFunctionType.Tanh
RELU = mybir.ActivationFunctionType.Relu

N_CORES = 8
B_LOC = 2
H = W = 64
HP = WP = 66
EG_ROWS = 16
CH_ROWS = 8
N_EG = H // EG_ROWS  # 4
N_MM = CH_ROWS * W  # 512
NE = 2 * N_MM  # 1024 elems per elementwise chunk


def _build(T=16):
    nc = bacc.Bacc("TRN2", target_bir_lowering=False, debug=False, num_devices=N_CORES)

    x9_d = nc.dram_tensor("x9", [T, 9, B_LOC, H, W], BF16, kind="ExternalInput").ap()
    w0p_d = nc.dram_tensor("w0p", [128, 3, 2, 128], BF16, kind="ExternalInput").ap()
    w0s_d = nc.dram_tensor("w0s", [64, 3, 2, 128], BF16, kind="ExternalInput").ap()
    w0x_d = nc.dram_tensor("w0x", [73, 2, 128], BF16, kind="ExternalInput").ap()
    w1_d = nc.dram_tensor("w1t", [128, 9, 2, 128], BF16, kind="ExternalInput").ap()
    b_d = nc.dram_tensor("bt", [128, 4], F32, kind="ExternalInput").ap()
    wh_d = nc.dram_tensor("wht", [128, 1], BF16, kind="ExternalInput").ap()
    bh_d = nc.dram_tensor("bht", [1, 1], F32, kind="ExternalInput").ap()
    y_d = nc.dram_tensor("y", [B_LOC, H * W], F32, kind="ExternalOutput").ap()

    with tile.TileContext(nc) as tc:
        with tc.tile_pool(name="state", bufs=1) as state, \
                tc.tile_pool(name="work", bufs=2) as work, \
                tc.tile_pool(name="psp", bufs=2, space="PSUM") as psp:
            # p0-63: h0 padded (base); p64-127: h0 shifted by (0,1)
            hh0 = state.tile([128, B_LOC, HP, WP], BF16)
            # p0-8: x(t) im2col; p9-72: h0 shifted by (2,2)
            x9h = state.tile([73, B_LOC, H, W], BF16)
            # p0-63: h0 padded (base, cell1 rhs); p64-127: h1 padded
            inp1 = state.tile([128, B_LOC, HP, WP], BF16)
            # p64-127: c state (f32); p0-63: tanh(g) scratch, free-aligned
            cg0 = state.tile([128, B_LOC, H * W], F32)
            cg1 = state.tile([128, B_LOC, H * W], F32)
            w0p = state.tile([128, 3, 2, 128], BF16)
            w0s = state.tile([64, 3, 2, 128], BF16)
            w0x = state.tile([73, 2, 128], BF16)
            w1t = state.tile([128, 9, 2, 128], BF16)
            b_sb = state.tile([128, 4], F32)
            whT = state.tile([128, 1], BF16)
            bh_sb = state.tile([1, 1], F32)

            nc.sync.dma_start(out=w0p, in_=w0p_d)
            nc.sync.dma_start(out=w0s, in_=w0s_d)
            nc.sync.dma_start(out=w0x, in_=w0x_d)
            nc.sync.dma_start(out=w1t, in_=w1_d)
            nc.sync.dma_start(out=b_sb, in_=b_d)
            nc.sync.dma_start(out=whT, in_=wh_d)
            nc.sync.dma_start(out=bh_sb, in_=bh_d)

            nc.vector.memset(hh0.bitcast(mybir.dt.uint16), 0)
            nc.vector.memset(x9h.bitcast(mybir.dt.uint16), 0)
            nc.vector.memset(inp1.bitcast(mybir.dt.uint16), 0)
            nc.vector.memset(cg0[64:128], 0.0)
            nc.vector.memset(cg1[64:128], 0.0)

            nc.sync.dma_start(out=x9h[0:9], in_=x9_d[0])

            h0w = {}   # (t, b, eg) -> cell0 h-write instr
            h1w = {}   # (t, b, eg) -> cell1 h-write instr
            mm_info = {}  # (t, b, eg) -> cell1 last-matmul instrs
            fan_info = {}  # (t, b) -> fanout dma instrs

            def conv0(t):
                """All cell0 conv matmuls for timestep t."""
                psums = {}
                for b in range(B_LOC):
                    for eg in range(N_EG):
                        p_if = psp.tile([128, 2, N_MM], F32, tag="pif", name=f"pif_{t}_0_{b}_{eg}")
                        p_og = psp.tile([128, 2, N_MM], F32, tag="pog", name=f"pog_{t}_0_{b}_{eg}")
                        psums[(b, eg)] = (p_if, p_og)
                        for half in range(2):
                            r0 = eg * EG_ROWS + half * CH_ROWS
                            for g, pt in ((0, p_if), (1, p_og)):
                                mms = []
                                # x im2col + h0@(2,2), K=73
                                mms.append(nc.tensor.matmul(
                                    pt[:, half], lhsT=w0x[:, g],
                                    rhs=x9h[0:73, b, r0:r0 + CH_ROWS, 0:W],
                                    start=True, stop=False))
                                # offset pairs (dy,0)+(dy,1), K=128
                                for dy in range(3):
                                    mms.append(nc.tensor.matmul(
                                        pt[:, half], lhsT=w0p[:, dy, g],
                                        rhs=hh0[0:128, b, r0 + dy:r0 + dy + CH_ROWS, 0:W],
                                        start=False, stop=False))
                                # singles (dy,2), dy=0,1, K=64
                                for dy in range(2):
                                    mms.append(nc.tensor.matmul(
                                        pt[:, half], lhsT=w0s[:, dy, g],
                                        rhs=hh0[0:64, b, r0 + dy:r0 + dy + CH_ROWS, 2:2 + W],
                                        start=False, stop=(dy == 1)))
                return psums

            def conv1(t):
                psums = {}
                for b in range(B_LOC):
                    for eg in range(N_EG):
                        info = mm_info.setdefault((t, b, eg), [])
                        p_if = psp.tile([128, 2, N_MM], F32, tag="pif", name=f"pif_{t}_1_{b}_{eg}")
                        p_og = psp.tile([128, 2, N_MM], F32, tag="pog", name=f"pog_{t}_1_{b}_{eg}")
                        psums[(b, eg)] = (p_if, p_og)
                        for half in range(2):
                            r0 = eg * EG_ROWS + half * CH_ROWS
                            for off in range(9):
                                dy, dx = off // 3, off % 3
                                rhs = inp1[0:128, b, r0 + dy:r0 + dy + CH_ROWS, dx:dx + W]
                                st, sp = off == 0, off == 8
                                mi = nc.tensor.matmul(p_if[:, half], lhsT=w1t[:, off, 0], rhs=rhs,
                                                      start=st, stop=sp)
                                mo = nc.tensor.matmul(p_og[:, half], lhsT=w1t[:, off, 1], rhs=rhs,
                                                      start=st, stop=sp)
                                if sp:
                                    info += [mi.ins, mo.ins]
                                # single-row seams into adjacent h0 chunks
                                if half == 0 and dy == 0 and (t, b, eg - 1) in h0w:
                                    for mm in (mi, mo):
                                        _add_dep_helper(mm.ins, h0w[(t, b, eg - 1)], reason="h0 seam RAW dn")
                                if half == 1 and dy == 2 and (t, b, eg + 1) in h0w:
                                    for mm in (mi, mo):
                                        _add_dep_helper(mm.ins, h0w[(t, b, eg + 1)], reason="h0 seam RAW up")
                return psums

            def elem(cell, t, psums):
                cg = cg0 if cell == 0 else cg1
                bcol = 2 * cell
                for b in range(B_LOC):
                    for eg in range(N_EG):
                        p_if, p_og = psums[(b, eg)]
                        pif_f = p_if.rearrange("p a b -> p (a b)")
                        pog_f = p_og.rearrange("p a b -> p (a b)")
                        cols = slice(eg * EG_ROWS * W, (eg + 1) * EG_ROWS * W)
                        cseg = cg[64:128, b, cols]
                        if_h = work.tile([128, NE], F32, tag="ifh", name=f"ifh_{t}_{cell}_{b}_{eg}")
                        g_h = work.tile([64, NE], F32, tag="gh", name=f"gh_{t}_{cell}_{b}_{eg}")
                        o_h = work.tile([64, NE], F32, tag="oh", name=f"oh_{t}_{cell}_{b}_{eg}")
                        t5 = work.tile([64, NE], F32, tag="t5", name=f"t5_{t}_{cell}_{b}_{eg}")
                        m1 = work.tile([128, NE], F32, tag="m1", name=f"m1_{t}_{cell}_{b}_{eg}")
                        m2 = work.tile([128, NE], F32, tag="m2", name=f"m2_{t}_{cell}_{b}_{eg}")
                        nc.scalar.activation(out=if_h, in_=pif_f, func=SIG,
                                             bias=b_sb[:, bcol:bcol + 1])
                        nc.scalar.activation(out=g_h, in_=pog_f[64:128], func=TANH,
                                             bias=b_sb[64:128, bcol + 1:bcol + 2])
                        nc.scalar.activation(out=o_h, in_=pog_f[0:64], func=SIG,
                                             bias=b_sb[0:64, bcol + 1:bcol + 2])
                        nc.vector.tensor_mul(m1[64:128], if_h[64:128], cseg)
                        nc.vector.tensor_mul(m2[64:128], if_h[0:64], g_h)
                        nc.vector.tensor_add(cseg, m1[64:128], m2[64:128])
                        nc.scalar.activation(out=t5, in_=cseg, func=TANH)
                        rows = slice(1 + eg * EG_ROWS, 1 + (eg + 1) * EG_ROWS)
                        if cell == 0:
                            hdst = inp1[0:64, b, rows, 1:1 + W]
                        else:
                            hdst = inp1[64:128, b, rows, 1:1 + W]
                        hw = nc.vector.tensor_mul(hdst, o_h, t5)
                        (h0w if cell == 0 else h1w)[(t, b, eg)] = hw.ins
                        if cell == 1:
                            for dg in (-1, 1):
                                if (t, b, eg + dg) in mm_info:
                                    for mm in mm_info[(t, b, eg + dg)]:
                                        _add_dep_helper(hw.ins, mm, reason="h1 seam WAR")
                    if cell == 0:
                        # fan out h0 to the shifted cell0-rhs copies; these
                        # overlap under cell1's conv matmuls
                        d1 = nc.sync.dma_start(out=hh0[0:64, b, 1:1 + H, 1:1 + W],
                                               in_=inp1[0:64, b, 1:1 + H, 1:1 + W])
                        d2 = nc.sync.dma_start(out=hh0[64:128, b, 1:1 + H, 0:W],
                                               in_=inp1[0:64, b, 1:1 + H, 1:1 + W])
                        d3 = nc.sync.dma_start(out=x9h[9:73, b, 0:H - 1, 0:W - 1],
                                               in_=inp1[0:64, b, 2:1 + H, 2:1 + W])
                        fan_info[(t, b)] = [d1.ins, d2.ins, d3.ins]

            for t in range(T):
                psums0 = conv0(t)
                if t + 1 < T:
                    nc.sync.dma_start(out=x9h[0:9], in_=x9_d[t + 1])
                elem(0, t, psums0)
                psums1 = conv1(t)
                elem(1, t, psums1)

            for b in range(B_LOC):
                for ch in range(H // CH_ROWS):
                    p_h = psp.tile([1, N_MM], F32, tag="pif", name=f"ph_{b}_{ch}")
                    rhs = inp1[64:128, b, 1 + ch * CH_ROWS:1 + (ch + 1) * CH_ROWS, 1:1 + W]
                    mh = nc.tensor.matmul(p_h, lhsT=whT[64:128], rhs=rhs, start=True, stop=True)
                    if (T - 1, b, ch // 2) in h1w:
                        _add_dep_helper(mh.ins, h1w[(T - 1, b, ch // 2)], reason="head RAW")
                    h_out = work.tile([1, N_MM], F32, tag="ho", name=f"ho_{b}_{ch}")
                    nc.scalar.activation(out=h_out, in_=p_h, func=RELU, bias=bh_sb[0:1, 0:1])
                    nc.sync.dma_start(out=y_d[b:b + 1, ch * N_MM:(ch + 1) * N_MM], in_=h_out)

    nc.compile()
    return nc


def _prep_inputs(x, w0, b0, w1, b1, wh, bh):
    bf16 = ml_dtypes.bfloat16
    x = np.asarray(x, np.float32)
    B, T = x.shape[0], x.shape[1]
    bl = B // N_CORES

    w0 = np.asarray(w0, np.float32).reshape(2, 128, 65, 3, 3)  # [g, m, k, dy, dx]
    # pairs: k<64 -> h-ch offset (dy,0); k>=64 -> h-ch offset (dy,1)
    w0p = np.zeros((128, 3, 2, 128), np.float32)
    for dy in range(3):
        for g in range(2):
            w0p[0:64, dy, g] = w0[g, :, 1:65, dy, 0].T
            w0p[64:128, dy, g] = w0[g, :, 1:65, dy, 1].T
    w0s = np.zeros((64, 3, 2, 128), np.float32)
    for dy in range(3):
        for g in range(2):
            w0s[:, dy, g] = w0[g, :, 1:65, dy, 2].T
    w0x = np.zeros((73, 2, 128), np.float32)
    for g in range(2):
        for o in range(9):
            w0x[o, g] = w0[g, :, 0, o // 3, o % 3]
        w0x[9:73, g] = w0[g, :, 1:65, 2, 2].T

    w1 = np.asarray(w1, np.float32).reshape(2, 128, 128, 3, 3)
    w1t = np.zeros((128, 9, 2, 128), np.float32)
    for o in range(9):
        for g in range(2):
            w1t[:, o, g] = w1[g, :, :, o // 3, o % 3].T

    b0 = np.asarray(b0, np.float32)
    b1 = np.asarray(b1, np.float32)
    bt = np.stack([b0[0:128], b0[128:256], b1[0:128], b1[128:256]], axis=1).astype(np.float32)
    wht = np.zeros((128, 1), np.float32)
    wht[64:128, 0] = np.asarray(wh, np.float32).reshape(64)
    bht = np.array([[float(np.asarray(bh).reshape(-1)[0])]], np.float32)

    # x im2col: x9[t, o, b, r, j] = xpad[b, t, r + dy, j + dx]
    xp_all = np.zeros((B, T, HP, WP), np.float32)
    xp_all[:, :, 1:1 + H, 1:1 + W] = x[:, :, 0]

    w0p = w0p.astype(bf16)
    w0s = w0s.astype(bf16)
    w0x = w0x.astype(bf16)
    w1t = w1t.astype(bf16)
    wht = wht.astype(bf16)

    in_maps = []
    for c in range(N_CORES):
        xp = xp_all[c * bl:(c + 1) * bl]  # [bl, T, 66, 66]
        x9 = np.zeros((T, 9, bl, H, W), np.float32)
        for o in range(9):
            dy, dx = o // 3, o % 3
            x9[:, o] = xp[:, :, dy:dy + H, dx:dx + W].transpose(1, 0, 2, 3)
        in_maps.append({"x9": np.ascontiguousarray(x9.astype(bf16)),
                        "w0p": w0p, "w0s": w0s, "w0x": w0x, "w1t": w1t,
                        "bt": bt, "wht": wht, "bht": bht})
    return in_maps


_NC_CACHE = {}


def kernel(x, w0, b0, w1, b1, wh, bh):
    x = np.asarray(x)
    B, T = x.shape[0], x.shape[1]
    if T not in _NC_CACHE:
        _NC_CACHE[T] = _build(T=T)
    nc = _NC_CACHE[T]
    in_maps = _prep_inputs(x, w0, b0, w1, b1, wh, bh)
    res = run_bass_kernel_spmd(nc, in_maps, core_ids=list(range(N_CORES)))
    bl = B // N_CORES
    out = np.zeros((B, 1, H, W), np.float32)
    for c, r in enumerate(res.results):
        out[c * bl:(c + 1) * bl, 0] = r["y"].reshape(bl, H, W)
    return out
